# revision 1
# baseline (speedup 1.0000x reference)
"""Trainium2 Bass kernel for BoundConvexViolationProjection.

Problem (hardcoded from the reference):
  x [32,8,512] f32, A [32,8,512,512] f32, b [32,8,512] f32, var_mask [32,512] f32 (ones)
  Iterate (up to MAX_ITER=100):
      r    = einsum('bsn,bsmn->bsm', x, A) - b
      viol = relu(r) - relu(-r - DELTA)
      g    = einsum('bsm,bsmn->bsn', viol, A)
      tv   = sum(relu(r), -1);  active = tv >= DELTA
      x    = max(where(active, x - LR*g/(|g|+EPS), x), 0)
  while any(active).  Key fact: per-(b,s) rows freeze once inactive (x stops
  changing => active stays false), so running the body a fixed MAX_ITER times
  with per-row gating is EXACTLY equivalent to the reference while_loop.

Sharding: data-parallel over batch B across 8 cores (4 batches = 32 (b,s)
pairs per core); the loop state is fully local, no collectives.

Per-core kernel strategy (memory-regime):
  Everything lives in the TRANSPOSED domain: state xT[n, pair], residual
  rT[m, pair], grad gT[n, pair] as dense PSUM/SBUF columns.  Both einsums
  are weight-stationary matmuls: the 128x128 A-block is the stationary
  operand (bf16 -> fast weight load), the x/viol column [128,1] is the
  moving operand, output is a [128,1] PSUM column accumulated over the
  contraction tiles.  This keeps every access pattern dense (engines
  can't address strided/offset partition starts) and eliminates all
  per-iteration transposes.
  - A^T (n-major) bf16 resident in SBUF (16 MiB) feeds the residual.
  - A (m-major) bf16: a few pairs resident, the rest streamed from HBM
    each iteration, feeding the grad matmuls.
  - Partition-direction reductions (tv, |g|^2) via ones-vector matmuls;
    the per-pair step scale is broadcast across partitions with a rank-1
    outer-product matmul.  Elementwise glue is [128, 64] dense on DVE/ACT.
  - 2-chunk (16-pair) software pipelining keeps PE busy and spreads DMA.
bf16 A with fp32 accumulation was validated against the f32 reference in
numpy: absmax relative error ~1.7e-4 over the full 100 iterations.
"""

import numpy as np
import ml_dtypes

import concourse.bacc as bacc
import concourse.bass as bass
import concourse.mybir as mybir
import concourse.tile as tile
from concourse.bass_utils import run_bass_kernel_spmd

BF16 = ml_dtypes.bfloat16

N_CORES = 8
B, S, M, N = 32, 8, 512, 512
B_LOC = B // N_CORES            # 4 batches per core
P = B_LOC * S                   # 32 (b,s) pairs per core
NT = N // 128                   # 4 n-tiles
MT = M // 128                   # 4 m-tiles
LR, DELTA = 0.005, 0.1
N_ITERS = 100
CPP = 8                         # pairs per pipeline chunk
NCH = P // CPP                  # 4 chunks
W = CPP * 4                     # 32 columns per chunk ((mt|nt, jj))
R_PER_CH = 3                    # resident row-layout pairs per chunk
N_SLOTS = 5                     # stream buffer slots (A rows, 512KiB each)


def _build_nc(n_iters=N_ITERS):
    f32 = mybir.dt.float32
    bf16 = mybir.dt.bfloat16
    Relu = mybir.ActivationFunctionType.Relu
    Sqrt = mybir.ActivationFunctionType.Sqrt
    Alu = mybir.AluOpType

    nc = bacc.Bacc("TRN2", target_bir_lowering=False)
    at_d = nc.dram_tensor("at", [P, 128, NT, 512], bf16, kind="ExternalInput")
    ar_d = nc.dram_tensor("arows", [P, 128, MT, 512], bf16, kind="ExternalInput")
    bt_d = nc.dram_tensor("bt", [128, NCH * W], f32, kind="ExternalInput")
    xt_d = nc.dram_tensor("x0t", [128, NCH * W], f32, kind="ExternalInput")
    id_d = nc.dram_tensor("ident", [128, 128], f32, kind="ExternalInput")
    out_d = nc.dram_tensor("xout", [P, 512], f32, kind="ExternalOutput")

    ones128 = nc.const_aps.tensor(1.0, (128, 1))  # [128,1] f32 ones (preamble)

    with tile.TileContext(nc) as tc:
        with (
            tc.tile_pool(name="resident", bufs=1) as res_pool,
            tc.tile_pool(name="stream", bufs=N_SLOTS) as stream_pool,
            tc.tile_pool(name="glue", bufs=7) as glue_pool,
            tc.tile_pool(name="violp", bufs=3) as viol_pool,
            tc.tile_pool(name="gpool", bufs=7) as g_pool,
            tc.tile_pool(name="xstate", bufs=2 * NCH + 2) as x_pool,
            tc.tile_pool(name="xtb", bufs=2 * NCH + 2) as xtb_pool,
            tc.tile_pool(name="rows", bufs=12) as row_pool,
            tc.tile_pool(name="mmps", bufs=5, space=bass.MemorySpace.PSUM) as mm_psum,
            tc.tile_pool(name="rowps", bufs=2, space=bass.MemorySpace.PSUM) as row_psum,
            tc.tile_pool(name="finps", bufs=1, space=bass.MemorySpace.PSUM) as fin_psum,
        ):
            # ---- persistent tiles + initial loads ----
            at_sb = res_pool.tile([128, P, NT, 512], bf16, tag="at_sb")
            ar_sb = res_pool.tile([128, NCH * R_PER_CH, MT, 512], bf16, tag="ar_sb")
            bt_sb = res_pool.tile([128, NCH * W], f32, tag="bt_sb")
            id_sb = res_pool.tile([128, 128], f32, tag="id_sb")
            cst = res_pool.tile([128, 2], f32, tag="cst")
            ones1 = res_pool.tile([1, 128], f32, tag="ones1")
            nc.vector.memset(cst[:, 0:1], -DELTA)
            nc.vector.memset(cst[:, 1:2], 1e-12)
            nc.vector.memset(ones1[:], 1.0)

            # resident pairs: first R_PER_CH of each chunk
            def res_idx(j):
                c, jj = divmod(j, CPP)
                return c * R_PER_CH + jj if jj < R_PER_CH else None

            # init loads via SWDGE (gpsimd): one shared semaphore, so any
            # compute op depending on them needs just one wait (walrus
            # allows a single sync-wait per compute instruction)
            for j in range(P):
                nc.gpsimd.dma_start(out=at_sb[:, j], in_=at_d[j])
                ri = res_idx(j)
                if ri is not None:
                    nc.gpsimd.dma_start(out=ar_sb[:, ri], in_=ar_d[j])
            nc.gpsimd.dma_start(out=bt_sb[:], in_=bt_d[:])
            nc.gpsimd.dma_start(out=id_sb[:], in_=id_d[:])

            x_cur = [None] * NCH    # f32 [128, W] transposed state per chunk
            xb_cur = [None] * NCH   # bf16 copy for matmul rhs
            slots = [dict() for _ in range(NCH)]
            pr_ps = [None] * NCH

            for c in range(NCH):
                xc = x_pool.tile([128, W], f32, tag="x")
                nc.gpsimd.dma_start(out=xc[:], in_=xt_d[:, c * W:(c + 1) * W])
                xb = xtb_pool.tile([128, W], bf16, tag="xb")
                nc.vector.tensor_copy(xb[:], xc[:])
                x_cur[c] = xc
                xb_cur[c] = xb

            # PE warm-up: one trash matmul depending on the LAST init load.
            # This folds the whole SWDGE init epoch into PE's vector clock,
            # so iteration-0 matmuls carry at most one other wait.
            warm = fin_psum.tile([1, 1], f32, tag="fin")
            nc.tensor.matmul(warm[:], x_cur[NCH - 1][:, 0:1],
                             x_cur[NCH - 1][:, 0:1], start=True, stop=True)

            def emit_res(c):
                # prefetch this chunk's streamed row-layout A (grad phase)
                sl = {}
                for j in range(c * CPP, (c + 1) * CPP):
                    if res_idx(j) is None:
                        t = stream_pool.tile([128, MT, 512], bf16, tag="slot")
                        nc.sync.dma_start(out=t[:], in_=ar_d[j])
                        sl[j] = t
                slots[c] = sl
                prg = mm_psum.tile([128, W], f32, tag="mm")
                xb = xb_cur[c]
                for jj in range(CPP):
                    j = c * CPP + jj
                    for mt in range(MT):
                        col = mt * CPP + jj
                        for nt in range(NT):
                            nc.tensor.matmul(
                                prg[:, col:col + 1],
                                at_sb[:, j, nt, mt * 128:(mt + 1) * 128],
                                xb[:, nt * CPP + jj: nt * CPP + jj + 1],
                                start=(nt == 0),
                                stop=(nt == NT - 1),
                            )
                pr_ps[c] = prg

            def emit_glue1(c):
                prg = pr_ps[c]
                r_sb = glue_pool.tile([128, W], f32, tag="glue")
                nc.vector.tensor_tensor(
                    r_sb[:], prg[:], bt_sb[:, c * W:(c + 1) * W], Alu.subtract)
                rp = glue_pool.tile([128, W], f32, tag="glue")
                nc.vector.tensor_scalar(out=rp[:], in0=r_sb[:], scalar1=0.0,
                                        scalar2=None, op0=Alu.max)
                r2 = glue_pool.tile([128, W], f32, tag="glue")
                nc.scalar.activation(r2[:], r_sb[:], Relu, scale=-1.0,
                                     bias=cst[:, 0:1])
                violT = viol_pool.tile([128, W], bf16, tag="viol")
                nc.vector.tensor_tensor(violT[:], rp[:], r2[:], Alu.subtract)
                return violT, rp

            def emit_tv(c, rp):
                # tv: column sums of relu(r) via ones-vector matmul.  Emitted
                # AFTER the violT write so its DVE wait also covers violT --
                # the grad matmuls then only wait on their stream DMA.
                tv4 = row_psum.tile([1, W], f32, tag="rowps")
                nc.tensor.matmul(tv4[:], ones128, rp[:], start=True, stop=True)
                tv = row_pool.tile([1, CPP], f32, tag="row")
                nc.vector.tensor_reduce(
                    tv[:],
                    tv4[:].rearrange("p (m j) -> p j m", j=CPP),
                    axis=mybir.AxisListType.X, op=Alu.add)
                mlr = row_pool.tile([1, CPP], f32, tag="row")
                nc.vector.tensor_scalar(out=mlr[:], in0=tv[:], scalar1=DELTA,
                                        scalar2=LR, op0=Alu.is_ge, op1=Alu.mult)
                return mlr

            def emit_grad(c, violT):
                pgg = mm_psum.tile([128, W], f32, tag="mm")
                for jj in range(CPP):
                    j = c * CPP + jj
                    ri = res_idx(j)
                    a_j = ar_sb[:, ri] if ri is not None else slots[c][j]
                    for nt in range(NT):
                        col = nt * CPP + jj
                        for mt in range(MT):
                            nc.tensor.matmul(
                                pgg[:, col:col + 1],
                                a_j[:, mt, nt * 128:(nt + 1) * 128],
                                violT[:, mt * CPP + jj: mt * CPP + jj + 1],
                                start=(mt == 0),
                                stop=(mt == MT - 1),
                            )
                return pgg

            def emit_gsq(c, pgg):
                gT = g_pool.tile([128, W], f32, tag="gt")
                nc.vector.tensor_copy(gT[:], pgg[:])
                sq = g_pool.tile([128, W], f32, tag="gt")
                nc.vector.tensor_tensor(sq[:], gT[:], gT[:], Alu.mult)
                return gT, sq

            def emit_sqmm(sq):
                s24 = row_psum.tile([1, W], f32, tag="rowps")
                nc.tensor.matmul(s24[:], ones128, sq[:], start=True, stop=True)
                return s24

            def emit_scale(mlr, s24):
                s2 = row_pool.tile([1, CPP], f32, tag="row")
                nc.vector.tensor_reduce(
                    s2[:],
                    s24[:].rearrange("p (m j) -> p j m", j=CPP),
                    axis=mybir.AxisListType.X, op=Alu.add)
                s = row_pool.tile([1, CPP], f32, tag="row")
                # sqrt(s2 + 1e-12): guards g==0 (reference adds EPS=1e-6 to
                # |g|; the difference is far below bf16 noise)
                nc.scalar.activation(s[:], s2[:], Sqrt, bias=cst[:1, 1:2])
                inv = row_pool.tile([1, CPP], f32, tag="row")
                nc.vector.reciprocal(inv[:], s[:])
                coef = row_pool.tile([1, CPP], f32, tag="row")
                nc.vector.tensor_tensor(coef[:], mlr[:], inv[:], Alu.mult)
                coef4 = row_pool.tile([1, W], f32, tag="row4")
                for nt in range(NT):
                    nc.vector.tensor_copy(coef4[:, nt * CPP:(nt + 1) * CPP],
                                          coef[:])
                return coef4

            def emit_outer(coef4):
                cb_ps = mm_psum.tile([128, W], f32, tag="mm")
                nc.tensor.matmul(cb_ps[:], ones1[:], coef4[:],
                                 start=True, stop=True)
                return cb_ps

            def emit_update(c, gT, cb_ps):
                cb = glue_pool.tile([128, W], f32, tag="glue")
                nc.vector.tensor_copy(cb[:], cb_ps[:])
                upd = glue_pool.tile([128, W], f32, tag="glue")
                nc.vector.tensor_tensor(upd[:], gT[:], cb[:], Alu.mult)
                xn = glue_pool.tile([128, W], f32, tag="glue")
                nc.vector.tensor_tensor(xn[:], x_cur[c][:], upd[:], Alu.subtract)
                xnew = x_pool.tile([128, W], f32, tag="x")
                nc.vector.tensor_scalar(out=xnew[:], in0=xn[:], scalar1=0.0,
                                        scalar2=None, op0=Alu.max)
                xb = xtb_pool.tile([128, W], bf16, tag="xb")
                nc.vector.tensor_copy(xb[:], xnew[:])
                x_cur[c] = xnew
                xb_cur[c] = xb

            # ---- main loop: software-pipelined chunk emission ----
            # PE emission order per step:  A(c) | SQ(c-2) | TV(c-1) G(c-1) |
            # OU(c-3), with DVE/ACT glue interleaved, so every aux matmul's
            # upstream DVE/ACT chain is hidden under a 256-MM res/grad block.
            pend_tvg = None   # (c, violT, mlr, rp)
            pend_sq = None    # (c, gT, mlr)
            pend_ou = None    # (c, gT, coef4)
            steps = n_iters * NCH
            for step in range(steps + 3):
                if step < steps:
                    c = step % NCH
                    emit_res(c)
                if pend_sq is not None:
                    sc, gT, mlr = pend_sq
                    s24 = emit_sqmm(gT[1])
                    coef4 = emit_scale(mlr, s24)
                    pend_ou2 = (sc, gT[0], coef4)
                else:
                    pend_ou2 = None
                if pend_tvg is not None:
                    tc_, violT, rp = pend_tvg
                    mlr = emit_tv(tc_, rp)
                    pgg = emit_grad(tc_, violT)
                    gTsq = emit_gsq(tc_, pgg)
                    pend_sq = (tc_, gTsq, mlr)
                else:
                    pend_sq = None
                if pend_ou is not None:
                    oc, gT0, coef4 = pend_ou
                    cb_ps = emit_outer(coef4)
                    emit_update(oc, gT0, cb_ps)
                pend_ou = pend_ou2
                if step < steps:
                    violT, rp = emit_glue1(c)
                    pend_tvg = (c, violT, rp)
                else:
                    pend_tvg = None

            # ---- store result: un-transpose once ----
            for c in range(NCH):
                pT = fin_psum.tile([W, 128], f32, tag="fin")
                nc.tensor.transpose(pT[:], x_cur[c][:], id_sb[:])
                fin = glue_pool.tile([W, 128], f32, tag="fin_sb")
                nc.vector.tensor_copy(fin[:], pT[:])
                for nt in range(NT):
                    nc.sync.dma_start(
                        out=out_d[c * CPP:(c + 1) * CPP,
                                  nt * 128:(nt + 1) * 128],
                        in_=fin[nt * CPP:(nt + 1) * CPP, :],
                    )

    nc.compile()
    return nc


_NC_CACHE = {}


def _get_nc(n_iters=N_ITERS):
    if n_iters not in _NC_CACHE:
        _NC_CACHE[n_iters] = _build_nc(n_iters)
    return _NC_CACHE[n_iters]


def _tcols(v):
    """[P, 512] -> [128, NCH*W] with col = c*W + t*CPP + jj, t = 128-block."""
    return np.ascontiguousarray(
        v.reshape(NCH, CPP, 4, 128).transpose(3, 0, 2, 1).reshape(128, NCH * W))


def _prep_core_inputs(Ac, bc, xc):
    """Ac [P,512,512] f32, bc [P,512], xc [P,512] -> per-core input map."""
    # at[j, p, nt, m] = Ac[j, m, nt*128+p]
    at = np.ascontiguousarray(
        Ac.reshape(P, M, NT, 128).transpose(0, 3, 2, 1)
    ).astype(BF16)
    # arows[j, p, mt, n] = Ac[j, mt*128+p, n]
    ar = np.ascontiguousarray(
        Ac.reshape(P, MT, 128, N).transpose(0, 2, 1, 3)
    ).astype(BF16)
    return {
        "at": at,
        "arows": ar,
        "bt": _tcols(np.asarray(bc, dtype=np.float32)),
        "x0t": _tcols(np.asarray(xc, dtype=np.float32)),
        "ident": np.eye(128, dtype=np.float32),
    }


def kernel(x, A, b, var_mask):
    x = np.asarray(x, dtype=np.float32)
    A = np.asarray(A, dtype=np.float32)
    b = np.asarray(b, dtype=np.float32)
    var_mask = np.asarray(var_mask, dtype=np.float32)

    nc = _get_nc()
    in_maps = []
    for c in range(N_CORES):
        bs = slice(c * B_LOC, (c + 1) * B_LOC)
        in_maps.append(
            _prep_core_inputs(
                A[bs].reshape(P, M, N), b[bs].reshape(P, M), x[bs].reshape(P, N)
            )
        )

    res = run_bass_kernel_spmd(nc, in_maps, list(range(N_CORES)))

    out = np.empty((B, S, N), dtype=np.float32)
    for c in range(N_CORES):
        out[c * B_LOC:(c + 1) * B_LOC] = res.results[c]["xout"].reshape(B_LOC, S, N)
    # reference returns x_fin * var_mask (var_mask is ones per the input spec;
    # this also keeps the general contract for any mask values)
    out *= var_mask[:, None, :]
    return out



# revision 14
# speedup vs baseline: 1.5425x; 1.5425x over previous
"""Trainium2 Bass kernel for BoundConvexViolationProjection (fp8 DoubleRow).

Problem (hardcoded from the reference):
  x [32,8,512] f32, A [32,8,512,512] f32, b [32,8,512] f32, var_mask [32,512]
  Iterate (MAX_ITER=100):
      r    = einsum('bsn,bsmn->bsm', x, A) - b
      viol = relu(r) - relu(-r - DELTA)
      g    = einsum('bsm,bsmn->bsn', viol, A)
      tv   = sum(relu(r), -1);  active = tv >= DELTA
      x    = max(where(active, x - LR*g/(|g|+EPS), x), 0)
  while any(active).  Rows freeze once inactive, so a fixed 100-iteration
  loop with per-row gating is exactly equivalent to the while_loop.

Sharding: data-parallel over batch B across 8 cores; 32 (b,s) pairs/core.

Per-core strategy (v2, fp8 DoubleRow):
  The baseline was LDWEIGHTS-bound (1024 weight loads/iter for 1-wide
  matvecs) plus 10 MiB/iter HBM streaming.  This version flips the
  operands: the per-pair state vector (x or viol) is the *stationary*
  operand (a [128,2,1] fp8 DoubleRow column, ~free to load) and the
  pair's A matrix is the 1024-wide fp8 *moving* operand.  One DR matmul
  contracts K=256 over 512 output columns in ~256 PE cycles, so one
  einsum for one pair is 2 matmuls -> 128 matmuls/iter total.  Both fp8
  A layouts (n-major for the residual, m-major for the grad) stay
  SBUF-resident (8 MiB each): the loop does zero HBM traffic.

  Matmul outputs are PSUM *rows* ([1,512] per pair).  A row can't be
  placed at an arbitrary base partition (tile_position is 32-aligned),
  so pairs are emitted in descending order with a widening stationary
  bundle q8[:, :, 0:jj+1]: the matmul writes rows 0..jj (start=True
  reclaims them), row jj is pair jj's result, and rows above survive
  from earlier (larger-jj) matmuls.  Matmul cost is free-dim bound, so
  the extra rows are free; 16 pairs stack into one [16,512] PSUM bank.

  Glue runs in row space [16,512] on DVE/ACT (tensor_tensor_reduce
  fuses tv / |g|^2 with their elementwise ops; per-partition scalar APs
  do the normalize/gate without broadcast matmuls).  x and viol return
  to fp8 stationary columns via 4 PE transposes + 4 DVE packs each.

  The 32 pairs run as two independent 16-pair halves, software-
  pipelined so each half's DVE/ACT glue hides under the other half's
  32-matmul PE block; half B's x-update is carried across the iteration
  boundary so PE never waits on DVE:
    PE order/iter: R_A xtB' R_B vtA G_A vtB G_B xtA   (128 MM + 16 tr)

fp8-e4m3 everywhere was validated against the f32 reference in numpy
(quantizing A both layouts, x and viol per iteration): max rel err
3.0e-3 over 100 iterations, vs the 2e-2 gate and bf16's 1.8e-4.
"""

import numpy as np
import ml_dtypes

import concourse.bacc as bacc
import concourse.bass as bass
import concourse.mybir as mybir
import concourse.tile as tile
from concourse.bass_utils import run_bass_kernel_spmd

F8 = ml_dtypes.float8_e4m3

N_CORES = 8
B, S, M, N = 32, 8, 512, 512
B_LOC = B // N_CORES            # 4 batches per core
P = B_LOC * S                   # 32 (b,s) pairs per core
H = P // 2                      # 16 pairs per half-phase
LR, DELTA = 0.005, 0.1
N_ITERS = 100


def _build_nc(n_iters=N_ITERS):
    f32 = mybir.dt.float32
    fp8 = mybir.dt.float8e4
    Relu = mybir.ActivationFunctionType.Relu
    Sqrt = mybir.ActivationFunctionType.Sqrt
    Square = mybir.ActivationFunctionType.Square
    Alu = mybir.AluOpType
    DR = mybir.MatmulPerfMode.DoubleRow

    nc = bacc.Bacc("TRN2", target_bir_lowering=False)
    at_d = nc.dram_tensor("at8", [P, 128, 2048], fp8, kind="ExternalInput")
    ar_d = nc.dram_tensor("ar8", [P, 128, 2048], fp8, kind="ExternalInput")
    b_d = nc.dram_tensor("brows", [P, 512], f32, kind="ExternalInput")
    x_d = nc.dram_tensor("x0rows", [P, 512], f32, kind="ExternalInput")
    id_d = nc.dram_tensor("ident", [H, H], f32, kind="ExternalInput")
    out_d = nc.dram_tensor("xout", [P, 512], f32, kind="ExternalOutput")

    with tile.TileContext(nc) as tc:
        with (
            tc.tile_pool(name="resident", bufs=1) as res_pool,
            tc.tile_pool(name="glue", bufs=14) as glue_pool,
            tc.tile_pool(name="rowv", bufs=12) as row_pool,
            tc.tile_pool(name="xsA", bufs=2) as xA_pool,
            tc.tile_pool(name="xsB", bufs=2) as xB_pool,
            tc.tile_pool(name="xqA", bufs=2) as xqA_pool,
            tc.tile_pool(name="xqB", bufs=2) as xqB_pool,
            tc.tile_pool(name="vqA", bufs=2) as vqA_pool,
            tc.tile_pool(name="vqB", bufs=2) as vqB_pool,
            tc.tile_pool(name="rps", bufs=2, space=bass.MemorySpace.PSUM) as r_psum,
            tc.tile_pool(name="gps", bufs=2, space=bass.MemorySpace.PSUM) as g_psum,
            tc.tile_pool(name="tps", bufs=4, space=bass.MemorySpace.PSUM) as t_psum,
        ):
            # ---- persistent tiles + initial loads (SWDGE via gpsimd) ----
            at_sb = res_pool.tile([128, P, 2048], fp8, tag="at_sb")
            ar_sb = res_pool.tile([128, P, 2048], fp8, tag="ar_sb")
            b_sb0 = res_pool.tile([H, 512], f32, tag="b0")
            b_sb1 = res_pool.tile([H, 512], f32, tag="b1")
            b_sb = [b_sb0, b_sb1]
            id_sb = res_pool.tile([H, H], f32, tag="id_sb")
            nd_sb = res_pool.tile([H, 1], f32, tag="nd_sb")
            eps_sb = res_pool.tile([H, 1], f32, tag="eps_sb")
            nc.vector.memset(nd_sb[:], -DELTA)
            nc.vector.memset(eps_sb[:], 1e-12)

            for j in range(P):
                nc.gpsimd.dma_start(out=at_sb[:, j], in_=at_d[j])
                nc.gpsimd.dma_start(out=ar_sb[:, j], in_=ar_d[j])
            for h in (0, 1):
                nc.gpsimd.dma_start(out=b_sb[h][:], in_=b_d[h * H:(h + 1) * H])
            nc.gpsimd.dma_start(out=id_sb[:], in_=id_d[:])

            x_cur = [None, None]
            for h, pool in ((0, xB_pool), (1, xA_pool)):
                xt = pool.tile([H, 512], f32, tag="x")
                nc.gpsimd.dma_start(out=xt[:], in_=x_d[h * H:(h + 1) * H])
                x_cur[h] = xt

            def emit_pack(src_rows, dst_q):
                """rows [H,512] -> 4 PE transposes + DVE fp8 packs.

                dst_q[k, a, t, jj] = src[jj, 256a + 128t + k] quantized."""
                tps = t_psum.tile([128, 4 * H], f32, tag="tp")
                for blk in range(4):
                    nc.tensor.transpose(
                        tps[:, blk * H:(blk + 1) * H],
                        src_rows[:, blk * 128:(blk + 1) * 128],
                        id_sb[:],
                    )
                for blk in range(4):
                    nc.vector.tensor_copy(
                        dst_q[:, blk // 2, blk % 2, :],
                        tps[:, blk * H:(blk + 1) * H])

            # initial fp8 stationaries
            xq_cur = [None, None]
            for h, pool in ((0, xqB_pool), (1, xqA_pool)):
                q = pool.tile([128, 2, 2, H], fp8, tag="xq")
                emit_pack(x_cur[h], q)
                xq_cur[h] = q

            def emit_mm_half(ps, q8, a_sb, h):
                """DR matmuls for half h, local pairs jj = H-1 .. 0."""
                for jj in range(H - 1, -1, -1):
                    j = h * H + jj
                    for k2 in range(2):
                        nc.tensor.matmul(
                            ps[0:jj + 1, :],
                            q8[:, k2, :, 0:jj + 1],
                            a_sb[:, j].rearrange("p (a t m) -> p a t m",
                                                 a=2, t=2)[:, k2],
                            start=(k2 == 0),
                            stop=(k2 == 1),
                            perf_mode=DR,
                        )

            def emit_glue1(r_ps, h):
                """viol + step coeff gate for half h; returns (viol, mlr)."""
                r_sb = glue_pool.tile([H, 512], f32, tag="glue")
                nc.vector.tensor_tensor(r_sb[:], r_ps[:], b_sb[h][:],
                                        Alu.subtract)
                rp = glue_pool.tile([H, 512], f32, tag="glue")
                tv = row_pool.tile([H, 1], f32, tag="row")
                nc.scalar.activation(rp[:], r_sb[:], Relu, accum_out=tv[:])
                r2 = glue_pool.tile([H, 512], f32, tag="glue")
                nc.scalar.activation(r2[:], r_sb[:], Relu, scale=-1.0,
                                     bias=nd_sb[:])
                viol = glue_pool.tile([H, 512], f32, tag="glue")
                nc.vector.tensor_tensor(viol[:], rp[:], r2[:], Alu.subtract)
                mlr = row_pool.tile([H, 1], f32, tag="row")
                nc.vector.tensor_scalar(out=mlr[:], in0=tv[:], scalar1=DELTA,
                                        scalar2=LR, op0=Alu.is_ge, op1=Alu.mult)
                return viol, mlr

            def emit_glue2(g_ps, mlr, x_prev, x_pool_h):
                """x <- max(x - mlr/|g| * g, 0) for one half; returns x_new."""
                gsq = glue_pool.tile([H, 512], f32, tag="glue")
                s2 = row_pool.tile([H, 1], f32, tag="row")
                nc.scalar.activation(gsq[:], g_ps[:], Square,
                                     accum_out=s2[:])
                s = row_pool.tile([H, 1], f32, tag="row")
                nc.scalar.activation(s[:], s2[:], Sqrt, bias=eps_sb[:])
                sinv = row_pool.tile([H, 1], f32, tag="row")
                nc.vector.reciprocal(sinv[:], s[:])
                coef = row_pool.tile([H, 1], f32, tag="row")
                nc.vector.tensor_tensor(coef[:], mlr[:], sinv[:], Alu.mult)
                upd = glue_pool.tile([H, 512], f32, tag="glue")
                nc.vector.tensor_scalar(out=upd[:], in0=g_ps[:],
                                        scalar1=coef[:], scalar2=None,
                                        op0=Alu.mult)
                xm = glue_pool.tile([H, 512], f32, tag="glue")
                nc.vector.tensor_tensor(xm[:], x_prev[:], upd[:], Alu.subtract)
                x_new = x_pool_h.tile([H, 512], f32, tag="x")
                nc.vector.tensor_scalar(out=x_new[:], in0=xm[:], scalar1=0.0,
                                        scalar2=None, op0=Alu.max)
                return x_new

            # ---- main loop: halves software-pipelined; B's x-update is
            # carried into the next iteration so PE never waits on DVE ----
            carryB = None   # (g_psB, mlrB) pending from previous iteration
            for it in range(n_iters):
                r_ps = r_psum.tile([H, 512], f32, tag="rps")
                emit_mm_half(r_ps, xq_cur[1], at_sb, 1)           # R_A
                if carryB is not None:                            # finish B
                    g_prev, mlr_prev = carryB
                    x_cur[0] = emit_glue2(g_prev, mlr_prev, x_cur[0], xB_pool)
                    q = xqB_pool.tile([128, 2, 2, H], fp8, tag="xq")
                    emit_pack(x_cur[0], q)                 # xtB on PE
                    xq_cur[0] = q
                r_psB = r_psum.tile([H, 512], f32, tag="rps")
                emit_mm_half(r_psB, xq_cur[0], at_sb, 0)          # R_B
                violA, mlrA = emit_glue1(r_ps, 1)                 # DVE ∥ R_B
                vqA = vqA_pool.tile([128, 2, 2, H], fp8, tag="vq")
                emit_pack(violA, vqA)

                g_ps = g_psum.tile([H, 512], f32, tag="gps")
                emit_mm_half(g_ps, vqA, ar_sb, 1)                 # G_A
                violB, mlrB = emit_glue1(r_psB, 0)                # DVE ∥ G_A
                vqB = vqB_pool.tile([128, 2, 2, H], fp8, tag="vq")
                emit_pack(violB, vqB)

                g_psB = g_psum.tile([H, 512], f32, tag="gps")
                emit_mm_half(g_psB, vqB, ar_sb, 0)                # G_B
                x_cur[1] = emit_glue2(g_ps, mlrA, x_cur[1], xA_pool)
                q = xqA_pool.tile([128, 2, 2, H], fp8, tag="xq")
                emit_pack(x_cur[1], q)                     # DVE ∥ G_B
                xq_cur[1] = q
                carryB = (g_psB, mlrB)

            # epilogue: final B-half update, then store rows straight out
            g_prev, mlr_prev = carryB
            x_cur[0] = emit_glue2(g_prev, mlr_prev, x_cur[0], xB_pool)
            for h in (0, 1):
                nc.sync.dma_start(out=out_d[h * H:(h + 1) * H],
                                  in_=x_cur[h][:])

    nc.compile()
    return nc


_NC_CACHE = {}


def _get_nc(n_iters=N_ITERS):
    if n_iters not in _NC_CACHE:
        _NC_CACHE[n_iters] = _build_nc(n_iters)
    return _NC_CACHE[n_iters]


def _prep_core_inputs(Ac, bc, xc):
    """Ac [P,512,512] f32, bc [P,512], xc [P,512] -> per-core input map."""
    # at8[j, k, nt2, t, m] = Ac[j, m, 256*nt2 + 128*t + k]  (n-major)
    at = np.ascontiguousarray(
        Ac.reshape(P, M, 2, 2, 128).transpose(0, 4, 2, 3, 1)
    ).astype(F8).reshape(P, 128, 2048)
    # ar8[j, k, mt2, t, n] = Ac[j, 256*mt2 + 128*t + k, n]  (m-major)
    ar = np.ascontiguousarray(
        Ac.reshape(P, 2, 2, 128, N).transpose(0, 3, 1, 2, 4)
    ).astype(F8).reshape(P, 128, 2048)
    return {
        "at8": at,
        "ar8": ar,
        "brows": np.ascontiguousarray(bc, dtype=np.float32),
        "x0rows": np.ascontiguousarray(xc, dtype=np.float32),
        "ident": np.eye(H, dtype=np.float32),
    }


def kernel(x, A, b, var_mask):
    x = np.asarray(x, dtype=np.float32)
    A = np.asarray(A, dtype=np.float32)
    b = np.asarray(b, dtype=np.float32)
    var_mask = np.asarray(var_mask, dtype=np.float32)

    nc = _get_nc()
    in_maps = []
    for c in range(N_CORES):
        bs = slice(c * B_LOC, (c + 1) * B_LOC)
        in_maps.append(
            _prep_core_inputs(
                A[bs].reshape(P, M, N), b[bs].reshape(P, M), x[bs].reshape(P, N)
            )
        )

    res = run_bass_kernel_spmd(nc, in_maps, list(range(N_CORES)))

    out = np.empty((B, S, N), dtype=np.float32)
    for c in range(N_CORES):
        out[c * B_LOC:(c + 1) * B_LOC] = res.results[c]["xout"].reshape(B_LOC, S, N)
    # reference returns x_fin * var_mask (ones per the input spec; kept for
    # the general contract)
    out *= var_mask[:, None, :]
    return out
